# revision 4
# baseline (speedup 1.0000x reference)
"""VQ codebook lookup kernel for Trainium2 (8 NeuronCores, data-parallel).

out[b] = emb[argmin_k ||x[b] - emb[k]||^2]

Per core (8192 rows of x):
  score[b,k] = 2*x.e_k - |e_k|^2  (argmax == argmin of distance)
  PE per 128-row tile: fp16 main product xh.eh (4 matmuls) plus the two
  precision corrections xl.eh + xh.el as fp8-e4m3 DoubleRow matmuls
  (256-deep contraction each, 2 instructions per K-half) -- half the
  cycles of an fp16 correction pass. fp8 operands are exponent-rebalanced
  (xl*2^6 x eh*2^-6, el*2^5 x xh*2^-5) so products carry scale 1.
  The -|e_k|^2 bias is folded into the xl.eh DoubleRow chunk as 6 exact
  fp8 ladder rows (displacing xl-correction dims 250..255; harmless).
  argmax: DVE folds the K halves with tensor-tensor max (g, 512 wide),
  then MAX8 + FIND_INDEX8 on g; the which-half bit comes from an ACT
  Sign pass with accumulate over the top half (count of s<m), freeing
  the DVE from a second full-width scan. Winning rows are fetched with
  an indirect-DMA gather from HBM; outputs leave via the ScalarE ring.
"""
import os
import sys

import numpy as np
import ml_dtypes

for _p in ("/opt/trn_rl_repo", "/root/.axon_site/_ro/trn_rl_repo"):
    if os.path.isdir(_p) and _p not in sys.path:
        sys.path.append(_p)

import concourse.bass as bass
import concourse.tile as tile
from concourse import bacc, mybir
from concourse.bass_utils import run_bass_kernel_spmd

N_CORES = 8
B, D, K = 65536, 256, 1024
BC = B // N_CORES            # rows per core
TILE_B = 128
N_TILES = BC // TILE_B       # 64
FOLD = False                 # fold K halves before MAX8/FI8
# column chunking of the x loads: small first chunk so the PE starts early
CHUNK_BOUNDS = [0, 256, 1024, 2048, 3072, 4096, 5120, 6144, 7168, 8192]

f32 = mybir.dt.float32
f16 = mybir.dt.float16
f8e4 = mybir.dt.float8e4
u32 = mybir.dt.uint32
E4NP = ml_dtypes.float8_e4m3

_nc_cache = {}


def _build():
    key = ("nc", FOLD)
    if key in _nc_cache:
        return _nc_cache[key]
    nc = bacc.Bacc()

    xh0 = nc.declare_dram_parameter("xh0", [128, BC], f16, isOutput=False)
    xh1 = nc.declare_dram_parameter("xh1", [128, BC], f16, isOutput=False)
    xl8 = nc.declare_dram_parameter("xl8", [128, 2, BC], f8e4, isOutput=False)
    xh8 = nc.declare_dram_parameter("xh8", [128, 2, BC], f8e4, isOutput=False)
    eh0 = nc.declare_dram_parameter("eh0", [128, K], f16, isOutput=False)
    eh1 = nc.declare_dram_parameter("eh1", [128, K], f16, isOutput=False)
    eh8d = nc.declare_dram_parameter("eh8d", [128, 2, K], f8e4, isOutput=False)
    el8d = nc.declare_dram_parameter("el8d", [128, 2, K], f8e4, isOutput=False)
    emb = nc.declare_dram_parameter("emb", [K, D], f32, isOutput=False)
    out = nc.declare_dram_parameter("out", [BC, D], f32, isOutput=True)

    with tile.TileContext(nc) as tc:
        with tc.tile_pool(name="res", bufs=1) as res, \
             tc.tile_pool(name="wrk", bufs=12) as wrk, \
             tc.tile_pool(name="ps", bufs=2, space="PSUM") as ps, \
             tc.tile_pool(name="psg", bufs=2, space="PSUM") as psg:
            teh0 = res.tile([128, K], f16, tag="eh0")
            teh1 = res.tile([128, K], f16, tag="eh1")
            teh8 = res.tile([128, 2, K], f8e4, tag="eh8")
            tel8 = res.tile([128, 2, K], f8e4, tag="el8")

            xch = {}
            xsrc = {"xh0": xh0, "xh1": xh1}
            x8src = {"xl8": xl8, "xh8": xh8}

            def load_chunk(nm, j):
                lo, hi = CHUNK_BOUNDS[j], CHUNK_BOUNDS[j + 1]
                if nm in xsrc:
                    t = res.tile([128, hi - lo], f16, tag=f"{nm}_{j}",
                                 name=f"{nm}_{j}")
                    nc.sync.dma_start(t[:], xsrc[nm][:, lo:hi])
                else:
                    t = res.tile([128, 2, hi - lo], f8e4, tag=f"{nm}_{j}",
                                 name=f"{nm}_{j}")
                    nc.sync.dma_start(t[:], x8src[nm][:, :, lo:hi])
                xch[(nm, j)] = t

            # load order mirrors the in-tile matmul order
            load_chunk("xh0", 0)
            nc.sync.dma_start(teh0[:], eh0[:])
            load_chunk("xh1", 0)
            nc.sync.dma_start(teh1[:], eh1[:])
            load_chunk("xl8", 0)
            nc.sync.dma_start(teh8[:], eh8d[:])
            load_chunk("xh8", 0)
            nc.sync.dma_start(tel8[:], el8d[:])
            for j in range(1, len(CHUNK_BOUNDS) - 1):
                for nm in ("xh0", "xh1", "xl8", "xh8"):
                    load_chunk(nm, j)

            def col(i):
                c0 = i * TILE_B
                for j in range(len(CHUNK_BOUNDS) - 1):
                    if c0 < CHUNK_BOUNDS[j + 1]:
                        return j, c0 - CHUNK_BOUNDS[j]
                raise AssertionError

            DR = mybir.MatmulPerfMode.DoubleRow

            for i in range(N_TILES):
                j, c0 = col(i)
                s = slice(c0, c0 + TILE_B)
                cxh0 = xch[("xh0", j)][:, s]
                cxh1 = xch[("xh1", j)][:, s]
                cxl8 = xch[("xl8", j)][:, :, s]
                cxh8 = xch[("xh8", j)][:, :, s]

                psc = ps.tile([128, K], f32, tag="scores")
                mm = nc.tensor.matmul
                for h in range(2):
                    hs = psc[:, h * 512:(h + 1) * 512]
                    ehs = slice(h * 512, (h + 1) * 512)
                    mm(hs, lhsT=cxh0, rhs=teh0[:, ehs], start=True, stop=False)
                    mm(hs, lhsT=cxh1, rhs=teh1[:, ehs], start=False, stop=False)
                    mm(hs, lhsT=cxl8, rhs=teh8[:, :, ehs], start=False,
                       stop=False, perf_mode=DR)
                    mm(hs, lhsT=cxh8, rhs=tel8[:, :, ehs], start=False,
                       stop=True, perf_mode=DR)

                tmax = wrk.tile([128, 8], f32, tag="maxv")
                tidx = wrk.tile([128, 8], u32, tag="idx")
                if FOLD:
                    g = psg.tile([128, 512], f32, tag="fold")
                    nc.vector.tensor_tensor(
                        out=g[:], in0=psc[:, 0:512], in1=psc[:, 512:1024],
                        op=mybir.AluOpType.max)
                    nc.vector.max(out=tmax[:], in_=g[:])
                    nc.vector.max_index(out=tidx[:], in_max=tmax[:],
                                        in_values=g[:])
                    # which-half bit: cnt = #{k in h1 : s_k < m} (512 or 511)
                    ohs = wrk.tile([128, 512], f16, tag="ohs")
                    cnt = wrk.tile([128, 1], f32, tag="cnt")
                    nc.scalar.activation(
                        out=ohs[:], in_=psc[:, 512:1024],
                        func=mybir.ActivationFunctionType.Sign,
                        bias=tmax[:, 0:1], scale=-1.0, accum_out=cnt[:])
                    # k* = j* + 512*(512 - cnt) = j* + (262144 - 512*cnt)
                    jf = wrk.tile([128, 1], f32, tag="jf")
                    nc.vector.tensor_scalar_add(out=jf[:], in0=tidx[:, 0:1],
                                                scalar1=0.0)
                    kf = wrk.tile([128, 1], f32, tag="kf")
                    nc.vector.scalar_tensor_tensor(
                        out=kf[:], in0=cnt[:], scalar=-512.0, in1=jf[:],
                        op0=mybir.AluOpType.mult, op1=mybir.AluOpType.add)
                    kidx = wrk.tile([128, 1], u32, tag="kidx")
                    nc.vector.tensor_scalar_add(out=kidx[:], in0=kf[:],
                                                scalar1=262144.0)
                    idx_ap = kidx[:]
                else:
                    nc.vector.max(out=tmax[:], in_=psc[:])
                    nc.vector.max_index(out=tidx[:], in_max=tmax[:],
                                        in_values=psc[:])
                    idx_ap = tidx[:, 0:1]

                tg = wrk.tile([128, D], f32, tag="gat")
                nc.gpsimd.indirect_dma_start(
                    out=tg[:],
                    out_offset=None,
                    in_=emb[:],
                    in_offset=bass.IndirectOffsetOnAxis(ap=idx_ap, axis=0),
                )
                nc.scalar.dma_start(out[i * TILE_B:(i + 1) * TILE_B, :], tg[:])

    nc.compile()
    _nc_cache[key] = nc
    return nc


def _bias_rows(q):
    """Exact 6-row fp8 ladder for the per-codeword bias q (shape [K])."""
    rows_x, rows_e = [], []
    r = q.astype(np.float64).copy()
    a = 4.0
    for _ in range(6):
        ej = np.asarray(r / a, np.float32).astype(E4NP)
        rows_x.append(a)
        rows_e.append(ej)
        r = r - a * ej.astype(np.float64)
        a = a / 16.0
        while np.abs(r).max() / a < 15.0 and a > 2.0 ** -20:
            a = a / 2.0
    return rows_x, rows_e


def _prepare_inputs(x, emb):
    x = np.ascontiguousarray(np.asarray(x, dtype=np.float32))
    emb = np.ascontiguousarray(np.asarray(emb, dtype=np.float32))

    e2 = np.ascontiguousarray(2.0 * emb.T).astype(np.float32)   # [D, K]
    eh = e2.astype(np.float16)
    el = (e2 - eh.astype(np.float32)).astype(np.float32)

    esq = (emb.astype(np.float64) ** 2).sum(axis=1)
    q = (-esq).astype(np.float32)

    xh = x.astype(np.float16)
    xl = (x - xh.astype(np.float32)).astype(np.float32)
    xhT = np.ascontiguousarray(xh.T)                            # [D, B] f16
    xlT = xl.T                                                  # [D, B] f32
    xhTf = xh.astype(np.float32).T                              # [D, B] f32

    # fp8 DoubleRow packs: [128 part, 2 half, N] with d = half*128 + part
    def drpack(a):                                              # [256, N] -> [128,2,N]
        return np.ascontiguousarray(
            a.reshape(2, 128, -1).transpose(1, 0, 2))

    xl8 = (xlT * 64.0).astype(E4NP)                             # [256, B]
    xh8 = (xhTf / 32.0).astype(E4NP)
    eh8 = (e2 / 64.0).astype(E4NP)                              # [256, K]
    el8 = (el * 32.0).astype(E4NP)

    # bias ladder rows displace xl-correction dims 250..255
    rows_x, rows_e = _bias_rows(q)
    xl8[250:256, :] = np.array(rows_x, E4NP)[:, None]
    eh8[250:256, :] = np.stack(rows_e).astype(E4NP)

    xl8p_full = drpack(xl8)                                     # [128,2,B]
    xh8p_full = drpack(xh8)
    eh8p = drpack(eh8)                                          # [128,2,K]
    el8p = drpack(el8)

    in_maps = []
    for c in range(N_CORES):
        sl = slice(c * BC, (c + 1) * BC)
        in_maps.append({
            "xh0": np.ascontiguousarray(xhT[:128, sl]),
            "xh1": np.ascontiguousarray(xhT[128:, sl]),
            "xl8": np.ascontiguousarray(xl8p_full[:, :, sl]),
            "xh8": np.ascontiguousarray(xh8p_full[:, :, sl]),
            "eh0": np.ascontiguousarray(eh[:128]),
            "eh1": np.ascontiguousarray(eh[128:]),
            "eh8d": eh8p,
            "el8d": el8p,
            "emb": emb,
        })
    return in_maps


def run(x, emb, trace=False, **kwargs):
    """Run the kernel; returns (out, BassKernelResults)."""
    nc = _build()
    in_maps = _prepare_inputs(x, emb)
    res = run_bass_kernel_spmd(nc, in_maps, list(range(N_CORES)),
                               trace=trace, **kwargs)
    out = np.concatenate([res.results[c]["out"] for c in range(N_CORES)], axis=0)
    return out, res


def kernel(x, emb):
    out, _ = run(x, emb, trace=False)
    return out


# revision 5
# speedup vs baseline: 1.3740x; 1.3740x over previous
"""VQ codebook lookup kernel for Trainium2 (8 NeuronCores, data-parallel).

out[b] = emb[argmin_k ||x[b] - emb[k]||^2]

Per core (8192 rows of x):
  score[b,k] = 2*x.e_k - |e_k|^2  (argmax == argmin of distance)
  PE per 128-row tile: fp16 main product xh.eh (4 matmuls) plus the two
  precision corrections xl.eh + xh.el as fp8-e4m3 DoubleRow matmuls
  (256-deep contraction each, 2 instructions per K-half) -- half the
  cycles of an fp16 correction pass. fp8 operands are exponent-rebalanced
  (xl*2^6 x eh*2^-6, el*2^5 x xh*2^-5) so products carry scale 1.
  The -|e_k|^2 bias is folded into the xl.eh DoubleRow chunk as 6 exact
  fp8 ladder rows (displacing xl-correction dims 250..255; harmless).
  argmax: DVE folds the K halves with tensor-tensor max (g, 512 wide),
  then MAX8 + FIND_INDEX8 on g; the which-half bit comes from an ACT
  Sign pass with accumulate over the top half (count of s<m), freeing
  the DVE from a second full-width scan. Winning rows are fetched with
  an indirect-DMA gather from HBM; outputs leave via the ScalarE ring.
"""
import os
import sys

import numpy as np
import ml_dtypes

for _p in ("/opt/trn_rl_repo", "/root/.axon_site/_ro/trn_rl_repo"):
    if os.path.isdir(_p) and _p not in sys.path:
        sys.path.append(_p)

import concourse.bass as bass
import concourse.tile as tile
from concourse import bacc, mybir
from concourse.bass_utils import run_bass_kernel_spmd

N_CORES = 8
B, D, K = 65536, 256, 1024
BC = B // N_CORES            # rows per core
TILE_B = 128
N_TILES = BC // TILE_B       # 64
FOLD = False                 # fold K halves before MAX8/FI8
# column chunking of the x loads: small first chunk so the PE starts early
CHUNK_BOUNDS = [0, 256, 1024, 2048, 3072, 4096, 5120, 6144, 7168, 8192]

f32 = mybir.dt.float32
f16 = mybir.dt.float16
f8e4 = mybir.dt.float8e4
u32 = mybir.dt.uint32
E4NP = ml_dtypes.float8_e4m3

_nc_cache = {}


def _build():
    key = ("nc", FOLD)
    if key in _nc_cache:
        return _nc_cache[key]
    nc = bacc.Bacc()

    xh0 = nc.declare_dram_parameter("xh0", [128, BC], f16, isOutput=False)
    xh1 = nc.declare_dram_parameter("xh1", [128, BC], f16, isOutput=False)
    xl8 = nc.declare_dram_parameter("xl8", [128, 2, BC], f8e4, isOutput=False)
    xh8 = nc.declare_dram_parameter("xh8", [128, 2, BC], f8e4, isOutput=False)
    eh0 = nc.declare_dram_parameter("eh0", [128, K], f16, isOutput=False)
    eh1 = nc.declare_dram_parameter("eh1", [128, K], f16, isOutput=False)
    eh8d = nc.declare_dram_parameter("eh8d", [128, 2, K], f8e4, isOutput=False)
    el8d = nc.declare_dram_parameter("el8d", [128, 2, K], f8e4, isOutput=False)
    emb = nc.declare_dram_parameter("emb", [K, D], f32, isOutput=False)
    out = nc.declare_dram_parameter("out", [BC, D], f32, isOutput=True)

    with tile.TileContext(nc) as tc:
        with tc.tile_pool(name="res", bufs=1) as res, \
             tc.tile_pool(name="wrk", bufs=12) as wrk, \
             tc.tile_pool(name="ps", bufs=3, space="PSUM") as ps, \
             tc.tile_pool(name="psg", bufs=1, space="PSUM") as psg:
            teh0 = res.tile([128, K], f16, tag="eh0")
            teh1 = res.tile([128, K], f16, tag="eh1")
            teh8 = res.tile([128, 2, K], f8e4, tag="eh8")
            tel8 = res.tile([128, 2, K], f8e4, tag="el8")

            xch = {}
            xsrc = {"xh0": xh0, "xh1": xh1}
            x8src = {"xl8": xl8, "xh8": xh8}

            def load_chunk(nm, j):
                lo, hi = CHUNK_BOUNDS[j], CHUNK_BOUNDS[j + 1]
                if nm in xsrc:
                    t = res.tile([128, hi - lo], f16, tag=f"{nm}_{j}",
                                 name=f"{nm}_{j}")
                    nc.sync.dma_start(t[:], xsrc[nm][:, lo:hi])
                else:
                    t = res.tile([128, 2, hi - lo], f8e4, tag=f"{nm}_{j}",
                                 name=f"{nm}_{j}")
                    nc.sync.dma_start(t[:], x8src[nm][:, :, lo:hi])
                xch[(nm, j)] = t

            # load order mirrors the in-tile matmul order
            load_chunk("xh0", 0)
            nc.sync.dma_start(teh0[:], eh0[:])
            load_chunk("xh1", 0)
            nc.sync.dma_start(teh1[:], eh1[:])
            load_chunk("xl8", 0)
            nc.sync.dma_start(teh8[:], eh8d[:])
            load_chunk("xh8", 0)
            nc.sync.dma_start(tel8[:], el8d[:])
            for j in range(1, len(CHUNK_BOUNDS) - 1):
                for nm in ("xh0", "xh1", "xl8", "xh8"):
                    load_chunk(nm, j)

            def col(i):
                c0 = i * TILE_B
                for j in range(len(CHUNK_BOUNDS) - 1):
                    if c0 < CHUNK_BOUNDS[j + 1]:
                        return j, c0 - CHUNK_BOUNDS[j]
                raise AssertionError

            DR = mybir.MatmulPerfMode.DoubleRow

            for i in range(N_TILES):
                j, c0 = col(i)
                s = slice(c0, c0 + TILE_B)
                cxh0 = xch[("xh0", j)][:, s]
                cxh1 = xch[("xh1", j)][:, s]
                cxl8 = xch[("xl8", j)][:, :, s]
                cxh8 = xch[("xh8", j)][:, :, s]

                psc = ps.tile([128, K], f32, tag="scores")
                mm = nc.tensor.matmul
                for h in range(2):
                    hs = psc[:, h * 512:(h + 1) * 512]
                    ehs = slice(h * 512, (h + 1) * 512)
                    mm(hs, lhsT=cxh0, rhs=teh0[:, ehs], start=True, stop=False)
                    mm(hs, lhsT=cxh1, rhs=teh1[:, ehs], start=False, stop=False)
                    mm(hs, lhsT=cxl8, rhs=teh8[:, :, ehs], start=False,
                       stop=False, perf_mode=DR)
                    mm(hs, lhsT=cxh8, rhs=tel8[:, :, ehs], start=False,
                       stop=True, perf_mode=DR)

                tmax = wrk.tile([128, 8], f32, tag="maxv")
                tidx = wrk.tile([128, 8], u32, tag="idx")
                if FOLD:
                    g = psg.tile([128, 512], f32, tag="fold")
                    nc.vector.tensor_tensor(
                        out=g[:], in0=psc[:, 0:512], in1=psc[:, 512:1024],
                        op=mybir.AluOpType.max)
                    nc.vector.max(out=tmax[:], in_=g[:])
                    nc.vector.max_index(out=tidx[:], in_max=tmax[:],
                                        in_values=g[:])
                    # which-half bit: cnt = #{k in h1 : s_k < m} (512 or 511)
                    ohs = wrk.tile([128, 512], f16, tag="ohs")
                    cnt = wrk.tile([128, 1], f32, tag="cnt")
                    nc.scalar.activation(
                        out=ohs[:], in_=psc[:, 512:1024],
                        func=mybir.ActivationFunctionType.Sign,
                        bias=tmax[:, 0:1], scale=-1.0, accum_out=cnt[:])
                    # k* = j* + 512*(512 - cnt) = j* + (262144 - 512*cnt)
                    jf = wrk.tile([128, 1], f32, tag="jf")
                    nc.vector.tensor_scalar_add(out=jf[:], in0=tidx[:, 0:1],
                                                scalar1=0.0)
                    kf = wrk.tile([128, 1], f32, tag="kf")
                    nc.vector.scalar_tensor_tensor(
                        out=kf[:], in0=cnt[:], scalar=-512.0, in1=jf[:],
                        op0=mybir.AluOpType.mult, op1=mybir.AluOpType.add)
                    kidx = wrk.tile([128, 1], u32, tag="kidx")
                    nc.vector.tensor_scalar_add(out=kidx[:], in0=kf[:],
                                                scalar1=262144.0)
                    idx_ap = kidx[:]
                else:
                    nc.vector.max(out=tmax[:], in_=psc[:])
                    nc.vector.max_index(out=tidx[:], in_max=tmax[:],
                                        in_values=psc[:])
                    idx_ap = tidx[:, 0:1]

                tg = wrk.tile([128, D], f32, tag="gat")
                nc.gpsimd.indirect_dma_start(
                    out=tg[:],
                    out_offset=None,
                    in_=emb[:],
                    in_offset=bass.IndirectOffsetOnAxis(ap=idx_ap, axis=0),
                )
                nc.scalar.dma_start(out[i * TILE_B:(i + 1) * TILE_B, :], tg[:])

    nc.compile()
    _nc_cache[key] = nc
    return nc


def _bias_rows(q):
    """Exact 6-row fp8 ladder for the per-codeword bias q (shape [K])."""
    rows_x, rows_e = [], []
    r = q.astype(np.float64).copy()
    a = 4.0
    for _ in range(6):
        ej = np.asarray(r / a, np.float32).astype(E4NP)
        rows_x.append(a)
        rows_e.append(ej)
        r = r - a * ej.astype(np.float64)
        a = a / 16.0
        while np.abs(r).max() / a < 15.0 and a > 2.0 ** -20:
            a = a / 2.0
    return rows_x, rows_e


def _prepare_inputs(x, emb):
    x = np.ascontiguousarray(np.asarray(x, dtype=np.float32))
    emb = np.ascontiguousarray(np.asarray(emb, dtype=np.float32))

    e2 = np.ascontiguousarray(2.0 * emb.T).astype(np.float32)   # [D, K]
    eh = e2.astype(np.float16)
    el = (e2 - eh.astype(np.float32)).astype(np.float32)

    esq = (emb.astype(np.float64) ** 2).sum(axis=1)
    q = (-esq).astype(np.float32)

    xh = x.astype(np.float16)
    xl = (x - xh.astype(np.float32)).astype(np.float32)
    xhT = np.ascontiguousarray(xh.T)                            # [D, B] f16
    xlT = xl.T                                                  # [D, B] f32
    xhTf = xh.astype(np.float32).T                              # [D, B] f32

    # fp8 DoubleRow packs: [128 part, 2 half, N] with d = half*128 + part
    def drpack(a):                                              # [256, N] -> [128,2,N]
        return np.ascontiguousarray(
            a.reshape(2, 128, -1).transpose(1, 0, 2))

    xl8 = (xlT * 64.0).astype(E4NP)                             # [256, B]
    xh8 = (xhTf / 32.0).astype(E4NP)
    eh8 = (e2 / 64.0).astype(E4NP)                              # [256, K]
    el8 = (el * 32.0).astype(E4NP)

    # bias ladder rows displace xl-correction dims 250..255
    rows_x, rows_e = _bias_rows(q)
    xl8[250:256, :] = np.array(rows_x, E4NP)[:, None]
    eh8[250:256, :] = np.stack(rows_e).astype(E4NP)

    xl8p_full = drpack(xl8)                                     # [128,2,B]
    xh8p_full = drpack(xh8)
    eh8p = drpack(eh8)                                          # [128,2,K]
    el8p = drpack(el8)

    in_maps = []
    for c in range(N_CORES):
        sl = slice(c * BC, (c + 1) * BC)
        in_maps.append({
            "xh0": np.ascontiguousarray(xhT[:128, sl]),
            "xh1": np.ascontiguousarray(xhT[128:, sl]),
            "xl8": np.ascontiguousarray(xl8p_full[:, :, sl]),
            "xh8": np.ascontiguousarray(xh8p_full[:, :, sl]),
            "eh0": np.ascontiguousarray(eh[:128]),
            "eh1": np.ascontiguousarray(eh[128:]),
            "eh8d": eh8p,
            "el8d": el8p,
            "emb": emb,
        })
    return in_maps


def run(x, emb, trace=False, **kwargs):
    """Run the kernel; returns (out, BassKernelResults)."""
    nc = _build()
    in_maps = _prepare_inputs(x, emb)
    res = run_bass_kernel_spmd(nc, in_maps, list(range(N_CORES)),
                               trace=trace, **kwargs)
    out = np.concatenate([res.results[c]["out"] for c in range(N_CORES)], axis=0)
    return out, res


def kernel(x, emb):
    out, _ = run(x, emb, trace=False)
    return out


# revision 6
# speedup vs baseline: 1.3908x; 1.0122x over previous
"""VQ codebook lookup kernel for Trainium2 (8 NeuronCores, data-parallel).

out[b] = emb[argmin_k ||x[b] - emb[k]||^2]

Per core (8192 rows of x):
  score[b,k] = 2*x.e_k - |e_k|^2  (argmax == argmin of distance)
  PE per 128-row tile: fp16 main product xh.eh (4 matmuls) plus the two
  precision corrections xl.eh + xh.el as fp8-e4m3 DoubleRow matmuls
  (256-deep contraction each, 2 instructions per K-half) -- half the
  cycles of an fp16 correction pass. fp8 operands are exponent-rebalanced
  (xl*2^6 x eh*2^-6, el*2^5 x xh*2^-5) so products carry scale 1.
  The -|e_k|^2 bias is folded into the xl.eh DoubleRow chunk as 6 exact
  fp8 ladder rows (displacing xl-correction dims 250..255; harmless).
  argmax: DVE folds the K halves with tensor-tensor max (g, 512 wide),
  then MAX8 + FIND_INDEX8 on g; the which-half bit comes from an ACT
  Sign pass with accumulate over the top half (count of s<m), freeing
  the DVE from a second full-width scan. Winning rows are fetched with
  an indirect-DMA gather from HBM; outputs leave via the ScalarE ring.
"""
import os
import sys

import numpy as np
import ml_dtypes

for _p in ("/opt/trn_rl_repo", "/root/.axon_site/_ro/trn_rl_repo"):
    if os.path.isdir(_p) and _p not in sys.path:
        sys.path.append(_p)

import concourse.bass as bass
import concourse.tile as tile
from concourse import bacc, mybir
from concourse.bass_utils import run_bass_kernel_spmd

N_CORES = 8
B, D, K = 65536, 256, 1024
BC = B // N_CORES            # rows per core
TILE_B = 128
N_TILES = BC // TILE_B       # 64
FOLD = False                 # fold K halves before MAX8/FI8
# column chunking of the x loads: small first chunk so the PE starts early
CHUNK_BOUNDS = [0, 256, 1024, 2048, 3072, 4096, 5120, 6144, 7168, 8192]

f32 = mybir.dt.float32
f16 = mybir.dt.float16
f8e4 = mybir.dt.float8e4
u32 = mybir.dt.uint32
E4NP = ml_dtypes.float8_e4m3

_nc_cache = {}


def _build():
    key = ("nc", FOLD)
    if key in _nc_cache:
        return _nc_cache[key]
    nc = bacc.Bacc()

    xh0 = nc.declare_dram_parameter("xh0", [128, BC], f16, isOutput=False)
    xh1 = nc.declare_dram_parameter("xh1", [128, BC], f16, isOutput=False)
    xl8 = nc.declare_dram_parameter("xl8", [128, 2, BC], f8e4, isOutput=False)
    xh8 = nc.declare_dram_parameter("xh8", [128, 2, BC], f8e4, isOutput=False)
    eh0 = nc.declare_dram_parameter("eh0", [128, K], f16, isOutput=False)
    eh1 = nc.declare_dram_parameter("eh1", [128, K], f16, isOutput=False)
    eh8d = nc.declare_dram_parameter("eh8d", [128, 2, K], f8e4, isOutput=False)
    el8d = nc.declare_dram_parameter("el8d", [128, 2, K], f8e4, isOutput=False)
    emb = nc.declare_dram_parameter("emb", [K, D], f32, isOutput=False)
    out = nc.declare_dram_parameter("out", [BC, D], f32, isOutput=True)

    with tile.TileContext(nc) as tc:
        with tc.tile_pool(name="res", bufs=1) as res, \
             tc.tile_pool(name="wrk", bufs=12) as wrk, \
             tc.tile_pool(name="ps", bufs=3, space="PSUM") as ps, \
             tc.tile_pool(name="psg", bufs=1, space="PSUM") as psg:
            teh0 = res.tile([128, K], f16, tag="eh0")
            teh1 = res.tile([128, K], f16, tag="eh1")
            teh8 = res.tile([128, 2, K], f8e4, tag="eh8")
            tel8 = res.tile([128, 2, K], f8e4, tag="el8")

            xch = {}
            xsrc = {"xh0": xh0, "xh1": xh1}
            x8src = {"xl8": xl8, "xh8": xh8}

            def load_chunk(nm, j):
                lo, hi = CHUNK_BOUNDS[j], CHUNK_BOUNDS[j + 1]
                if nm in xsrc:
                    t = res.tile([128, hi - lo], f16, tag=f"{nm}_{j}",
                                 name=f"{nm}_{j}")
                    nc.sync.dma_start(t[:], xsrc[nm][:, lo:hi])
                else:
                    t = res.tile([128, 2, hi - lo], f8e4, tag=f"{nm}_{j}",
                                 name=f"{nm}_{j}")
                    nc.sync.dma_start(t[:], x8src[nm][:, :, lo:hi])
                xch[(nm, j)] = t

            # x chunks stream on the Sync ring; e-tables on the Scalar ring
            # split by K-half so tile 0 can start after the half-A tables.
            load_chunk("xh0", 0)
            nc.scalar.dma_start(teh0[:, 0:512], eh0[:, 0:512])
            load_chunk("xh1", 0)
            nc.scalar.dma_start(teh1[:, 0:512], eh1[:, 0:512])
            load_chunk("xl8", 0)
            nc.scalar.dma_start(teh8[:, :, 0:512], eh8d[:, :, 0:512])
            load_chunk("xh8", 0)
            nc.scalar.dma_start(tel8[:, :, 0:512], el8d[:, :, 0:512])
            nc.scalar.dma_start(teh0[:, 512:1024], eh0[:, 512:1024])
            nc.scalar.dma_start(teh1[:, 512:1024], eh1[:, 512:1024])
            nc.scalar.dma_start(teh8[:, :, 512:1024], eh8d[:, :, 512:1024])
            nc.scalar.dma_start(tel8[:, :, 512:1024], el8d[:, :, 512:1024])
            for j in range(1, len(CHUNK_BOUNDS) - 1):
                for nm in ("xh0", "xh1", "xl8", "xh8"):
                    load_chunk(nm, j)

            def col(i):
                c0 = i * TILE_B
                for j in range(len(CHUNK_BOUNDS) - 1):
                    if c0 < CHUNK_BOUNDS[j + 1]:
                        return j, c0 - CHUNK_BOUNDS[j]
                raise AssertionError

            DR = mybir.MatmulPerfMode.DoubleRow

            for i in range(N_TILES):
                j, c0 = col(i)
                s = slice(c0, c0 + TILE_B)
                cxh0 = xch[("xh0", j)][:, s]
                cxh1 = xch[("xh1", j)][:, s]
                cxl8 = xch[("xl8", j)][:, :, s]
                cxh8 = xch[("xh8", j)][:, :, s]

                psc = ps.tile([128, K], f32, tag="scores")
                mm = nc.tensor.matmul
                for h in range(2):
                    hs = psc[:, h * 512:(h + 1) * 512]
                    ehs = slice(h * 512, (h + 1) * 512)
                    mm(hs, lhsT=cxh0, rhs=teh0[:, ehs], start=True, stop=False)
                    mm(hs, lhsT=cxh1, rhs=teh1[:, ehs], start=False, stop=False)
                    mm(hs, lhsT=cxl8, rhs=teh8[:, :, ehs], start=False,
                       stop=False, perf_mode=DR)
                    mm(hs, lhsT=cxh8, rhs=tel8[:, :, ehs], start=False,
                       stop=True, perf_mode=DR)

                tmax = wrk.tile([128, 8], f32, tag="maxv")
                tidx = wrk.tile([128, 8], u32, tag="idx")
                if FOLD:
                    g = psg.tile([128, 512], f32, tag="fold")
                    nc.vector.tensor_tensor(
                        out=g[:], in0=psc[:, 0:512], in1=psc[:, 512:1024],
                        op=mybir.AluOpType.max)
                    nc.vector.max(out=tmax[:], in_=g[:])
                    nc.vector.max_index(out=tidx[:], in_max=tmax[:],
                                        in_values=g[:])
                    # which-half bit: cnt = #{k in h1 : s_k < m} (512 or 511)
                    ohs = wrk.tile([128, 512], f16, tag="ohs")
                    cnt = wrk.tile([128, 1], f32, tag="cnt")
                    nc.scalar.activation(
                        out=ohs[:], in_=psc[:, 512:1024],
                        func=mybir.ActivationFunctionType.Sign,
                        bias=tmax[:, 0:1], scale=-1.0, accum_out=cnt[:])
                    # k* = j* + 512*(512 - cnt) = j* + (262144 - 512*cnt)
                    jf = wrk.tile([128, 1], f32, tag="jf")
                    nc.vector.tensor_scalar_add(out=jf[:], in0=tidx[:, 0:1],
                                                scalar1=0.0)
                    kf = wrk.tile([128, 1], f32, tag="kf")
                    nc.vector.scalar_tensor_tensor(
                        out=kf[:], in0=cnt[:], scalar=-512.0, in1=jf[:],
                        op0=mybir.AluOpType.mult, op1=mybir.AluOpType.add)
                    kidx = wrk.tile([128, 1], u32, tag="kidx")
                    nc.vector.tensor_scalar_add(out=kidx[:], in0=kf[:],
                                                scalar1=262144.0)
                    idx_ap = kidx[:]
                else:
                    nc.vector.max(out=tmax[:], in_=psc[:])
                    nc.vector.max_index(out=tidx[:], in_max=tmax[:],
                                        in_values=psc[:])
                    idx_ap = tidx[:, 0:1]

                tg = wrk.tile([128, D], f32, tag="gat")
                nc.gpsimd.indirect_dma_start(
                    out=tg[:],
                    out_offset=None,
                    in_=emb[:],
                    in_offset=bass.IndirectOffsetOnAxis(ap=idx_ap, axis=0),
                )
                nc.scalar.dma_start(out[i * TILE_B:(i + 1) * TILE_B, :], tg[:])

    nc.compile()
    _nc_cache[key] = nc
    return nc


def _bias_rows(q):
    """Exact 6-row fp8 ladder for the per-codeword bias q (shape [K])."""
    rows_x, rows_e = [], []
    r = q.astype(np.float64).copy()
    a = 4.0
    for _ in range(6):
        ej = np.asarray(r / a, np.float32).astype(E4NP)
        rows_x.append(a)
        rows_e.append(ej)
        r = r - a * ej.astype(np.float64)
        a = a / 16.0
        while np.abs(r).max() / a < 15.0 and a > 2.0 ** -20:
            a = a / 2.0
    return rows_x, rows_e


def _prepare_inputs(x, emb):
    x = np.ascontiguousarray(np.asarray(x, dtype=np.float32))
    emb = np.ascontiguousarray(np.asarray(emb, dtype=np.float32))

    e2 = np.ascontiguousarray(2.0 * emb.T).astype(np.float32)   # [D, K]
    eh = e2.astype(np.float16)
    el = (e2 - eh.astype(np.float32)).astype(np.float32)

    esq = (emb.astype(np.float64) ** 2).sum(axis=1)
    q = (-esq).astype(np.float32)

    xh = x.astype(np.float16)
    xl = (x - xh.astype(np.float32)).astype(np.float32)
    xhT = np.ascontiguousarray(xh.T)                            # [D, B] f16
    xlT = xl.T                                                  # [D, B] f32
    xhTf = xh.astype(np.float32).T                              # [D, B] f32

    # fp8 DoubleRow packs: [128 part, 2 half, N] with d = half*128 + part
    def drpack(a):                                              # [256, N] -> [128,2,N]
        return np.ascontiguousarray(
            a.reshape(2, 128, -1).transpose(1, 0, 2))

    xl8 = (xlT * 64.0).astype(E4NP)                             # [256, B]
    xh8 = (xhTf / 32.0).astype(E4NP)
    eh8 = (e2 / 64.0).astype(E4NP)                              # [256, K]
    el8 = (el * 32.0).astype(E4NP)

    # bias ladder rows displace xl-correction dims 250..255
    rows_x, rows_e = _bias_rows(q)
    xl8[250:256, :] = np.array(rows_x, E4NP)[:, None]
    eh8[250:256, :] = np.stack(rows_e).astype(E4NP)

    xl8p_full = drpack(xl8)                                     # [128,2,B]
    xh8p_full = drpack(xh8)
    eh8p = drpack(eh8)                                          # [128,2,K]
    el8p = drpack(el8)

    in_maps = []
    for c in range(N_CORES):
        sl = slice(c * BC, (c + 1) * BC)
        in_maps.append({
            "xh0": np.ascontiguousarray(xhT[:128, sl]),
            "xh1": np.ascontiguousarray(xhT[128:, sl]),
            "xl8": np.ascontiguousarray(xl8p_full[:, :, sl]),
            "xh8": np.ascontiguousarray(xh8p_full[:, :, sl]),
            "eh0": np.ascontiguousarray(eh[:128]),
            "eh1": np.ascontiguousarray(eh[128:]),
            "eh8d": eh8p,
            "el8d": el8p,
            "emb": emb,
        })
    return in_maps


def run(x, emb, trace=False, **kwargs):
    """Run the kernel; returns (out, BassKernelResults)."""
    nc = _build()
    in_maps = _prepare_inputs(x, emb)
    res = run_bass_kernel_spmd(nc, in_maps, list(range(N_CORES)),
                               trace=trace, **kwargs)
    out = np.concatenate([res.results[c]["out"] for c in range(N_CORES)], axis=0)
    return out, res


def kernel(x, emb):
    out, _ = run(x, emb, trace=False)
    return out
